# revision 20
# baseline (speedup 1.0000x reference)
"""CapsuleLayer (dynamic routing, ROUTING_ITER=2) Bass/Tile kernel for TRN2.

Contract: kernel(x, weight) takes FULL inputs
  x:      [64, 2048, 1, 16] f32
  weight: [1, 2048, 32, 16, 16] f32
returns FULL output [64, 32, 16] f32.

Sharding: data-parallel over batch B=64 across 8 cores (8 per core),
weight replicated. Self-contained: hardcodes shapes, imports only
numpy/ml_dtypes/concourse.

v3: replicated s via [128,128] delta lhsT (no DRAM bounces between
routing passes), software-pipelined routing stages, DVE/Pool split by
j (prod/tree) and d (y), 3-way PSUM->SBUF copy spread.
"""

from contextlib import ExitStack

import ml_dtypes
import numpy as np

import concourse.bacc as bacc
import concourse.bass as bass
import concourse.mybir as mybir
import concourse.tile as tile
from concourse.bass_utils import run_bass_kernel_spmd

F32 = mybir.dt.float32
BF16 = mybir.dt.bfloat16
AF = mybir.ActivationFunctionType
AX = mybir.AxisListType

EPS = 1e-8
J, D, E = 32, 16, 16
JD = J * D  # 512


def emit_capsule(tc, w2, xbd, dsrep, d1rep, out, n_in, b_loc=8):
    """Emit the per-core capsule program.

    DRAM tensors (APs):
      w2    [G, 8, E, JD] bf16  weight, host-permuted to [i, e, d, j], i=g*8+di
      xbd   [128, G, 64]  bf16  block-diag x stationary: [(di,e), g, (b,di')]
      dsrep [128, 128] bf16     replicating delta: [p, m] = (b(p) == b(m))
      d1rep [128, 128] bf16     dsrep / 32 (s1 accumulation)
      out   [b_loc, JD] f32     squash(s3) output, (d,j) free layout

    u layout in SBUF (bf16): partition p = (g%2)*64 + b*8 + di,
    free f = (g//2)*JD + d*J + j, where i = g*8 + di.
    s/V/v_exp are replicated: row p holds batch b(p) = (p//8)%8.
    """
    nc = tc.nc
    assert b_loc == 8
    G = n_in // 8
    GH = G // 2
    GDMA = 16  # groups per W DMA chunk
    CH = 16   # gh per routing chunk
    JS = 26   # j-columns handled by DVE in prod/tree (rest on Pool)
    DS = 13   # d-rows handled by DVE in y (rest on Pool)
    assert G % GDMA == 0 and GH % CH == 0
    NCH = GH // CH

    ctx = ExitStack()
    singles = ctx.enter_context(tc.tile_pool(name="singles", bufs=1))
    small = ctx.enter_context(tc.tile_pool(name="small", bufs=2))

    u_sb = singles.tile([128, GH * JD], BF16)
    ds_sb = singles.tile([128, 128], BF16)
    d1_sb = singles.tile([128, 128], BF16)
    nc.sync.dma_start(out=ds_sb, in_=dsrep)
    nc.sync.dma_start(out=d1_sb, in_=d1rep)
    v_exp = singles.tile([128, JD], BF16)
    V = singles.tile([128, JD], F32)     # running sum of v vectors (replicated)
    s_sb = singles.tile([128, JD], F32)
    eps1 = singles.tile([128, 1], F32)
    nc.vector.memset(eps1, EPS)

    # ---------- squash helpers on replicated [128, JD] tiles ----------
    def squash(s_in, v_out, axis_j):
        # squash over j (axis_j=True): sq[b,d] = sum_j s^2
        # squash over d (axis_j=False): sq[b,j] = sum_d s^2
        t2 = small.tile([128, JD], F32, tag="sqt2")
        nc.vector.tensor_mul(t2, s_in, s_in)
        n = D if axis_j else J
        sv = small.tile([128, 4, J], F32, tag="sqv")
        sq, a, t3, w = sv[:, 0, :n], sv[:, 1, :n], sv[:, 2, :n], sv[:, 3, :n]
        if axis_j:
            nc.vector.reduce_sum(out=sq, in_=t2.rearrange("p (d j) -> p d j", d=D),
                                 axis=AX.X)
        else:
            nc.vector.reduce_sum(out=sq, in_=t2.rearrange("p (d j) -> p j d", d=D),
                                 axis=AX.X)
        nc.scalar.activation(a, sq, AF.Sqrt, bias=eps1)
        nc.vector.tensor_mul(t3, sq, a)
        nc.vector.tensor_add(t3, t3, a)          # a*(1+sq)
        nc.vector.reciprocal(w, t3)
        nc.vector.tensor_mul(w, w, sq)           # sq/((1+sq)a)
        if axis_j:
            wb = w.unsqueeze(2).broadcast_to([128, D, J])
        else:
            wb = w.unsqueeze(1).broadcast_to([128, D, J])
        nc.vector.tensor_mul(v_out.rearrange("p (d j) -> p d j", d=D),
                             s_in.rearrange("p (d j) -> p d j", d=D), wb)

    # ---------- phase 1: W pass (u = W @ x) + replicated s1 chain ----------
    with tc.tile_pool(name="wp", bufs=3) as wp, \
         tc.tile_pool(name="xp", bufs=2) as xp, \
         tc.tile_pool(name="up", bufs=2, space="PSUM") as up, \
         tc.tile_pool(name="sp", bufs=1, space="PSUM") as sp:
        s1_ps = sp.tile([128, JD], F32)
        cp_eng = [nc.vector, nc.scalar]
        cpi = 0
        for ci in range(G // GDMA):
            wt = wp.tile([128, GDMA, JD], BF16, tag="wt")
            wsrc = w2[ci * GDMA:(ci + 1) * GDMA].rearrange("g di e f -> (di e) g f")
            nc.sync.dma_start(out=wt, in_=wsrc)
            xt = xp.tile([128, GDMA, 64], BF16, tag="xt")
            nc.sync.dma_start(out=xt, in_=xbd[:, ci * GDMA:(ci + 1) * GDMA])
            for gq in range(GDMA // 4):
                pt = up.tile([128, 2 * JD], F32, tag="upt")
                for idx in range(4):
                    gl = gq * 4 + idx
                    nc.tensor.matmul(
                        pt[(gl % 2) * 64:(gl % 2) * 64 + 64,
                           (idx // 2) * JD:(idx // 2) * JD + JD],
                        xt[:, gl, :], wt[:, gl, :], start=True, stop=True)
                gh0 = (ci * GDMA) // 2 + gq * 2
                dst = u_sb[:, gh0 * JD:(gh0 + 2) * JD]
                eng = cp_eng[cpi % 2]
                if eng is nc.scalar:
                    eng.copy(out=dst, in_=pt)
                else:
                    eng.tensor_copy(out=dst, in_=pt)
                cpi += 1
                for gh in (gh0, gh0 + 1):
                    nc.tensor.matmul(s1_ps, d1_sb,
                                     u_sb[:, gh * JD:(gh + 1) * JD],
                                     start=(gh == 0), stop=(gh == GH - 1))
        nc.vector.tensor_copy(out=s_sb, in_=s1_ps)

    squash(s_sb, V, axis_j=True)      # V = v1 (replicated f32)
    nc.vector.tensor_copy(out=v_exp, in_=V)

    # ---------- routing pass (T = u.V, softmax, s = sum_i c*u) ----------
    def routing_pass(final):
        with tc.tile_pool(name="rpd", bufs=1) as rpd, \
             tc.tile_pool(name="rpp", bufs=1) as rpp, \
             tc.tile_pool(name="rp", bufs=2) as rp, \
             tc.tile_pool(name="yp", bufs=2) as yp, \
             tc.tile_pool(name="cmb", bufs=1) as cmb, \
             tc.tile_pool(name="spp", bufs=1, space="PSUM") as spp:
            s_ps = spp.tile([128, 4, JD], F32)   # 4 interleaved chains
            JP = J - JS
            DP = D - DS
            v4 = v_exp.rearrange("p (d j) -> p d j", d=D)

            tts, cs = {}, {}

            def stage_a(k):
                # prod = u*v, tree-reduce over d -> tt
                gh0 = k * CH
                u4 = u_sb[:, gh0 * JD:(gh0 + CH) * JD].rearrange(
                    "p (g d j) -> p g d j", d=D, j=J)
                prod_d = rpd.tile([128, CH, D, JS], BF16, tag="prod_d")
                prod_p = rpp.tile([128, CH, D, JP], BF16, tag="prod_p")
                vbd = v4[:, :, 0:JS].unsqueeze(1).broadcast_to([128, CH, D, JS])
                vbp = v4[:, :, JS:J].unsqueeze(1).broadcast_to([128, CH, D, JP])
                nc.vector.tensor_mul(prod_d, u4[:, :, :, 0:JS], vbd)
                nc.gpsimd.tensor_mul(prod_p, u4[:, :, :, JS:J], vbp)
                tt = rp.tile([128, CH, J], BF16, tag="tt")
                tts[k] = tt

                def tree(eng, p4, nj, joff):
                    # in-place pairwise reduction over d into p4[:, :, 0, :]
                    eng.tensor_add(p4[:, :, 0:8, :], p4[:, :, 0:8, :],
                                   p4[:, :, 8:16, :])
                    eng.tensor_add(p4[:, :, 0:4, :], p4[:, :, 0:4, :],
                                   p4[:, :, 4:8, :])
                    eng.tensor_add(p4[:, :, 0:2, :], p4[:, :, 0:2, :],
                                   p4[:, :, 2:4, :])
                    eng.tensor_add(tt[:, :, joff:joff + nj].unsqueeze(2),
                                   p4[:, :, 0:1, :], p4[:, :, 1:2, :])

                tree(nc.vector, prod_d, JS, 0)
                tree(nc.gpsimd, prod_p, JP, JS)

            def stage_b(k):
                # softmax over j: eT = exp(tt); c = eT / sum_j eT
                tt = tts.pop(k)
                eT = rp.tile([128, CH, J], BF16, tag="eT")
                nc.scalar.activation(eT, tt, AF.Exp)
                se = rp.tile([128, CH], F32, tag="se")
                nc.vector.reduce_sum(out=se, in_=eT, axis=AX.X)
                r = rp.tile([128, CH], F32, tag="r")
                nc.vector.reciprocal(r, se)
                c = rp.tile([128, CH, J], BF16, tag="c")
                nc.gpsimd.tensor_mul(
                    c, eT, r.unsqueeze(2).broadcast_to([128, CH, J]))
                cs[k] = c

            def stage_c(k):
                # y = c (broadcast over d) * u ; s += sum_i y via dsrep matmul
                gh0 = k * CH
                u4 = u_sb[:, gh0 * JD:(gh0 + CH) * JD].rearrange(
                    "p (g d j) -> p g d j", d=D, j=J)
                c = cs.pop(k)
                y = yp.tile([128, CH, D, J], BF16, tag="y")
                cbd = c.unsqueeze(2).broadcast_to([128, CH, DS, J])
                nc.vector.tensor_mul(y[:, :, 0:DS], u4[:, :, 0:DS], cbd)
                cbp = c.unsqueeze(2).broadcast_to([128, CH, DP, J])
                nc.gpsimd.tensor_mul(y[:, :, DS:D], u4[:, :, DS:D], cbp)
                for q in range(CH):
                    gh = gh0 + q
                    nc.tensor.matmul(s_ps[:, gh % 4, :], ds_sb,
                                     y[:, q].rearrange("p d j -> p (d j)"),
                                     start=(gh < 4), stop=(gh >= GH - 4))

            for k in range(NCH + 2):
                if k < NCH:
                    stage_a(k)
                if 1 <= k <= NCH:
                    stage_b(k - 1)
                if 2 <= k:
                    stage_c(k - 2)
            # combine the 4 chains (only one PSUM operand allowed per op)
            A = cmb.tile([128, 2, JD], F32, tag="cmbA")
            nc.scalar.copy(out=A[:, 0, :], in_=s_ps[:, 1, :])
            nc.scalar.copy(out=A[:, 1, :], in_=s_ps[:, 3, :])
            nc.vector.tensor_add(A[:, 0, :], s_ps[:, 0, :], A[:, 0, :])
            nc.vector.tensor_add(A[:, 1, :], s_ps[:, 2, :], A[:, 1, :])
            nc.vector.tensor_add(s_sb, A[:, 0, :], A[:, 1, :])
        if not final:
            v2 = small.tile([128, JD], F32, tag="v2")
            squash(s_sb, v2, axis_j=True)
            nc.vector.tensor_add(V, V, v2)
            nc.vector.tensor_copy(out=v_exp, in_=V)
        else:
            vout = small.tile([128, JD], F32, tag="vout")
            squash(s_sb, vout, axis_j=False)
            for b in range(8):
                nc.sync.dma_start(out=out[b:b + 1, :], in_=vout[b * 8:b * 8 + 1, :])

    routing_pass(final=False)   # iteration 2 (uses V=v1)
    routing_pass(final=True)    # final (uses V=v1+v2)
    ctx.close()


def build_module(n_in=2048, b_loc=8, num_devices=8, enable_asserts=False):
    nc = bacc.Bacc("TRN2", target_bir_lowering=False, debug=False,
                   num_devices=num_devices, enable_asserts=enable_asserts)
    G = n_in // 8
    w2 = nc.dram_tensor("w2", [G, 8, E, JD], BF16, kind="ExternalInput").ap()
    xbd = nc.dram_tensor("xbd", [128, G, 64], BF16, kind="ExternalInput").ap()
    dsrep = nc.dram_tensor("dsrep", [128, 128], BF16, kind="ExternalInput").ap()
    d1rep = nc.dram_tensor("d1rep", [128, 128], BF16, kind="ExternalInput").ap()
    out = nc.dram_tensor("out", [b_loc, JD], F32, kind="ExternalOutput").ap()
    with tile.TileContext(nc) as tc:
        emit_capsule(tc, w2, xbd, dsrep, d1rep, out, n_in=n_in, b_loc=b_loc)
    nc.compile()
    return nc


def host_prep_w(weight, n_in):
    # weight [1, N, J, D, E] -> w2 [G, 8, E, J*D] with free layout (d, j)
    w2 = np.ascontiguousarray(weight[0].transpose(0, 3, 2, 1))  # [N, E, D, J]
    return w2.reshape(n_in // 8, 8, E, JD).astype(ml_dtypes.bfloat16)


def host_prep_xbd(xs, n_in):
    # xs [b_loc, N, E] -> xbd [128, G, 64] block-diagonal stationary, k-major
    G = n_in // 8
    t = xs.reshape(8, G, 8, E).transpose(1, 2, 3, 0)  # [G, di, e, b]
    xbd = np.zeros((G, 8, E, 8, 8), np.float32)       # [G, di, e, b, di']
    for di in range(8):
        xbd[:, di, :, :, di] = t[:, di]
    return np.ascontiguousarray(
        xbd.reshape(G, 128, 64).transpose(1, 0, 2)).astype(ml_dtypes.bfloat16)


def host_prep_deltas():
    p = np.arange(128)
    m = np.arange(128)
    mask = ((p[:, None] // 8) % 8) == ((m[None, :] // 8) % 8)
    dsrep = mask.astype(np.float32)
    d1rep = dsrep / 32.0
    return (dsrep.astype(ml_dtypes.bfloat16), d1rep.astype(ml_dtypes.bfloat16))


_CACHE = {}
LAST_EXEC_NS = None


def kernel(x, weight, trace=False):
    B, N_in = 64, 2048
    n_cores = 8
    b_loc = B // n_cores
    key = (N_in, b_loc, n_cores)
    if key not in _CACHE:
        _CACHE[key] = build_module(n_in=N_in, b_loc=b_loc, num_devices=n_cores)
    nc = _CACHE[key]

    x = np.asarray(x, dtype=np.float32)
    weight = np.asarray(weight, dtype=np.float32)
    w2 = host_prep_w(weight, N_in)
    dsrep, d1rep = host_prep_deltas()
    in_maps = []
    for c in range(n_cores):
        xs = np.ascontiguousarray(x[c * b_loc:(c + 1) * b_loc, :, 0, :])
        in_maps.append({
            "w2": w2,
            "xbd": host_prep_xbd(xs, N_in),
            "dsrep": dsrep,
            "d1rep": d1rep,
        })
    global LAST_EXEC_NS
    res = run_bass_kernel_spmd(nc, in_maps, core_ids=list(range(n_cores)),
                               trace=trace)
    LAST_EXEC_NS = res.exec_time_ns
    outs = [r["out"].reshape(b_loc, D, J).transpose(0, 2, 1) for r in res.results]
    return np.ascontiguousarray(np.concatenate(outs, axis=0))


# revision 41
# speedup vs baseline: 1.0987x; 1.0987x over previous
"""CapsuleLayer (dynamic routing, ROUTING_ITER=2) Bass/Tile kernel for TRN2.

Contract: kernel(x, weight) takes FULL inputs
  x:      [64, 2048, 1, 16] f32
  weight: [1, 2048, 32, 16, 16] f32
returns FULL output [64, 32, 16] f32.

Sharding: data-parallel over batch B=64 across 8 cores (8 per core),
weight replicated. Self-contained: hardcodes shapes, imports only
numpy/ml_dtypes/concourse.

v3: replicated s via [128,128] delta lhsT (no DRAM bounces between
routing passes), software-pipelined routing stages, DVE/Pool split by
j (prod/tree) and d (y), 3-way PSUM->SBUF copy spread.
"""

from contextlib import ExitStack

import ml_dtypes
import numpy as np

import concourse.bacc as bacc
import concourse.bass as bass
import concourse.mybir as mybir
import concourse.tile as tile
from concourse.bass_utils import run_bass_kernel_spmd

F32 = mybir.dt.float32
BF16 = mybir.dt.bfloat16
AF = mybir.ActivationFunctionType
AX = mybir.AxisListType

EPS = 1e-8
J, D, E = 32, 16, 16
JD = J * D  # 512


def emit_capsule(tc, w2, xbd, dsrep, d1rep, out, n_in, b_loc=8):
    """Emit the per-core capsule program.

    DRAM tensors (APs):
      w2    [G, 8, E, JD] bf16  weight, host-permuted to [i, e, d, j], i=g*8+di
      xbd   [128, G, 64]  bf16  block-diag x stationary: [(di,e), g, (b,di')]
      dsrep [128, 128] bf16     replicating delta: [p, m] = (b(p) == b(m))
      d1rep [128, 128] bf16     dsrep / 32 (s1 accumulation)
      out   [b_loc, JD] f32     squash(s3) output, (d,j) free layout

    u layout in SBUF (bf16): partition p = (g%2)*64 + b*8 + di,
    free f = (g//2)*JD + d*J + j, where i = g*8 + di.
    s/V/v_exp are replicated: row p holds batch b(p) = (p//8)%8.
    """
    nc = tc.nc
    assert b_loc == 8
    G = n_in // 8
    GH = G // 2
    GDMA = 16  # groups per W DMA chunk
    CH = 16   # gh per routing chunk
    JS = 28   # j-columns handled by DVE in prod/tree (rest on Pool)
    DS = 11   # d-rows handled by DVE in y (rest on Pool)
    assert G % GDMA == 0 and GH % CH == 0
    NCH = GH // CH

    ctx = ExitStack()
    singles = ctx.enter_context(tc.tile_pool(name="singles", bufs=1))
    small = ctx.enter_context(tc.tile_pool(name="small", bufs=2))

    u_sb = singles.tile([128, GH * JD], BF16)
    ds_sb = singles.tile([128, 128], BF16)
    d1_sb = singles.tile([128, 128], BF16)
    nc.sync.dma_start(out=ds_sb, in_=dsrep)
    nc.sync.dma_start(out=d1_sb, in_=d1rep)
    v_exp = singles.tile([128, JD], BF16)
    V = singles.tile([128, JD], F32)     # running sum of v vectors (replicated)
    s_sb = singles.tile([128, JD], F32)
    eps1 = singles.tile([128, 1], F32)
    nc.vector.memset(eps1, EPS)

    # ---------- squash helpers on replicated [128, JD] tiles ----------
    def squash(s_in, v_out, axis_j):
        # squash over j (axis_j=True): sq[b,d] = sum_j s^2
        # squash over d (axis_j=False): sq[b,j] = sum_d s^2
        t2 = small.tile([128, JD], F32, tag="sqt2")
        nc.vector.tensor_mul(t2, s_in, s_in)
        n = D if axis_j else J
        sv = small.tile([128, 4, J], F32, tag="sqv")
        sq, a, t3, w = sv[:, 0, :n], sv[:, 1, :n], sv[:, 2, :n], sv[:, 3, :n]
        if axis_j:
            nc.vector.reduce_sum(out=sq, in_=t2.rearrange("p (d j) -> p d j", d=D),
                                 axis=AX.X)
        else:
            nc.vector.reduce_sum(out=sq, in_=t2.rearrange("p (d j) -> p j d", d=D),
                                 axis=AX.X)
        nc.scalar.activation(a, sq, AF.Sqrt, bias=eps1)
        nc.vector.tensor_mul(t3, sq, a)
        nc.vector.tensor_add(t3, t3, a)          # a*(1+sq)
        nc.vector.reciprocal(w, t3)
        nc.vector.tensor_mul(w, w, sq)           # sq/((1+sq)a)
        if axis_j:
            wb = w.unsqueeze(2).broadcast_to([128, D, J])
        else:
            wb = w.unsqueeze(1).broadcast_to([128, D, J])
        nc.vector.tensor_mul(v_out.rearrange("p (d j) -> p d j", d=D),
                             s_in.rearrange("p (d j) -> p d j", d=D), wb)

    # ---------- phase 1: W pass (u = W @ x) + replicated s1 chain ----------
    with tc.tile_pool(name="wp", bufs=3) as wp, \
         tc.tile_pool(name="xp", bufs=2) as xp, \
         tc.tile_pool(name="up", bufs=2, space="PSUM") as up, \
         tc.tile_pool(name="sp", bufs=1, space="PSUM") as sp:
        s1_ps = sp.tile([128, JD], F32)
        cp_eng = [nc.vector, nc.scalar]
        cpi = 0
        s1_pend = []
        for ci in range(G // GDMA):
            wt = wp.tile([128, GDMA, JD], BF16, tag="wt")
            wsrc = w2[ci * GDMA:(ci + 1) * GDMA].rearrange("g di e f -> (di e) g f")
            nc.sync.dma_start(out=wt, in_=wsrc)
            xt = xp.tile([128, GDMA, 64], BF16, tag="xt")
            nc.sync.dma_start(out=xt, in_=xbd[:, ci * GDMA:(ci + 1) * GDMA])
            for gq in range(GDMA // 4):
                pt = up.tile([128, 2 * JD], F32, tag="upt")
                for idx in range(4):
                    gl = gq * 4 + idx
                    nc.tensor.matmul(
                        pt[(gl % 2) * 64:(gl % 2) * 64 + 64,
                           (idx // 2) * JD:(idx // 2) * JD + JD],
                        xt[:, gl, :], wt[:, gl, :], start=True, stop=True)
                gh0 = (ci * GDMA) // 2 + gq * 2
                dst = u_sb[:, gh0 * JD:(gh0 + 2) * JD]
                eng = cp_eng[cpi % 2]
                if eng is nc.scalar:
                    eng.copy(out=dst, in_=pt)
                else:
                    eng.tensor_copy(out=dst, in_=pt)
                cpi += 1
                s1_pend.extend((gh0, gh0 + 1))
                while len(s1_pend) > 2:
                    gh = s1_pend.pop(0)
                    nc.tensor.matmul(s1_ps, d1_sb,
                                     u_sb[:, gh * JD:(gh + 1) * JD],
                                     start=(gh == 0), stop=False)
        for gh in s1_pend:
            nc.tensor.matmul(s1_ps, d1_sb, u_sb[:, gh * JD:(gh + 1) * JD],
                             start=False, stop=(gh == GH - 1))
        nc.vector.tensor_copy(out=s_sb, in_=s1_ps)

    squash(s_sb, V, axis_j=True)      # V = v1 (replicated f32)
    nc.vector.tensor_copy(out=v_exp, in_=V)

    # ---------- routing pass (T = u.V, softmax, s = sum_i c*u) ----------
    def routing_pass(final):
        with tc.tile_pool(name="rpd", bufs=1) as rpd, \
             tc.tile_pool(name="rpp", bufs=1) as rpp, \
             tc.tile_pool(name="rp", bufs=3) as rp, \
             tc.tile_pool(name="yp", bufs=2) as yp, \
             tc.tile_pool(name="cmb", bufs=1) as cmb, \
             tc.tile_pool(name="spp", bufs=1, space="PSUM") as spp:
            s_ps = spp.tile([128, 1, JD], F32)   # single chain
            JP = J - JS
            DP = D - DS
            v4 = v_exp.rearrange("p (d j) -> p d j", d=D)

            tts, cs = {}, {}
            CHUNKS = [16]*7+[8,6,2]
            OFFS = []
            o = 0
            for ch in CHUNKS:
                OFFS.append(o)
                o += ch
            assert o == GH

            def stage_a(k):
                # prod = u*v, tree-reduce over d -> tt
                gh0, ch = OFFS[k], CHUNKS[k]
                u4 = u_sb[:, gh0 * JD:(gh0 + ch) * JD].rearrange(
                    "p (g d j) -> p g d j", d=D, j=J)
                prod_d = rpd.tile([128, CH, D, JS], BF16, tag="prod_d")
                prod_d = prod_d[:, 0:ch]
                prod_p = rpp.tile([128, CH, D, JP], BF16, tag="prod_p")
                prod_p = prod_p[:, 0:ch]
                vbd = v4[:, :, 0:JS].unsqueeze(1).broadcast_to([128, ch, D, JS])
                vbp = v4[:, :, JS:J].unsqueeze(1).broadcast_to([128, ch, D, JP])
                nc.vector.tensor_mul(prod_d, u4[:, :, :, 0:JS], vbd)
                nc.gpsimd.tensor_mul(prod_p, u4[:, :, :, JS:J], vbp)
                tt = rp.tile([128, CH, J], BF16, tag="tt")
                tt = tt[:, 0:ch]
                tts[k] = tt

                def tree(eng, p4, nj, joff):
                    # in-place pairwise reduction over d into p4[:, :, 0, :]
                    eng.tensor_add(p4[:, :, 0:8, :], p4[:, :, 0:8, :],
                                   p4[:, :, 8:16, :])
                    eng.tensor_add(p4[:, :, 0:4, :], p4[:, :, 0:4, :],
                                   p4[:, :, 4:8, :])
                    eng.tensor_add(p4[:, :, 0:2, :], p4[:, :, 0:2, :],
                                   p4[:, :, 2:4, :])
                    eng.tensor_add(tt[:, :, joff:joff + nj].unsqueeze(2),
                                   p4[:, :, 0:1, :], p4[:, :, 1:2, :])

                tree(nc.vector, prod_d, JS, 0)
                tree(nc.gpsimd, prod_p, JP, JS)

            def stage_b(k):
                # softmax over j: eT = exp(tt); c = eT / sum_j eT
                ch = CHUNKS[k]
                tt = tts.pop(k)
                eT = rp.tile([128, CH, J], BF16, tag="eT")
                eT = eT[:, 0:ch]
                nc.scalar.activation(eT, tt, AF.Exp)
                se = rp.tile([128, CH], F32, tag="se")
                se = se[:, 0:ch]
                nc.vector.reduce_sum(out=se, in_=eT, axis=AX.X)
                r = rp.tile([128, CH], F32, tag="r")
                r = r[:, 0:ch]
                nc.vector.reciprocal(r, se)
                c = rp.tile([128, CH, J], BF16, tag="c")
                c = c[:, 0:ch]
                nc.gpsimd.tensor_mul(
                    c, eT, r.unsqueeze(2).broadcast_to([128, ch, J]))
                cs[k] = c

            def stage_c(k):
                # y = c (broadcast over d) * u ; s += sum_i y via dsrep matmul
                gh0, ch = OFFS[k], CHUNKS[k]
                u4 = u_sb[:, gh0 * JD:(gh0 + ch) * JD].rearrange(
                    "p (g d j) -> p g d j", d=D, j=J)
                c = cs.pop(k)
                y = yp.tile([128, CH, D, J], BF16, tag="y")
                y = y[:, 0:ch]
                cbd = c.unsqueeze(2).broadcast_to([128, ch, DS, J])
                nc.vector.tensor_mul(y[:, :, 0:DS], u4[:, :, 0:DS], cbd)
                cbp = c.unsqueeze(2).broadcast_to([128, ch, DP, J])
                nc.gpsimd.tensor_mul(y[:, :, DS:D], u4[:, :, DS:D], cbp)
                for q in range(ch):
                    gh = gh0 + q
                    nc.tensor.matmul(s_ps[:, 0, :], ds_sb,
                                     y[:, q].rearrange("p d j -> p (d j)"),
                                     start=(gh == 0), stop=(gh == GH - 1))

            NC = len(CHUNKS)
            for k in range(NC + 2):
                if k < NC:
                    stage_a(k)
                if 1 <= k <= NC:
                    stage_b(k - 1)
                if 2 <= k:
                    stage_c(k - 2)
            nc.vector.tensor_copy(out=s_sb, in_=s_ps[:, 0, :])
        if not final:
            v2 = small.tile([128, JD], F32, tag="vtmp")
            squash(s_sb, v2, axis_j=True)
            nc.vector.tensor_add(V, V, v2)
            nc.vector.tensor_copy(out=v_exp, in_=V)
        else:
            vout = small.tile([128, JD], F32, tag="vtmp")
            squash(s_sb, vout, axis_j=False)
            for b in range(8):
                nc.sync.dma_start(out=out[b:b + 1, :], in_=vout[b * 8:b * 8 + 1, :])

    routing_pass(final=False)   # iteration 2 (uses V=v1)
    routing_pass(final=True)    # final (uses V=v1+v2)
    ctx.close()


def build_module(n_in=2048, b_loc=8, num_devices=8, enable_asserts=False):
    nc = bacc.Bacc("TRN2", target_bir_lowering=False, debug=False,
                   num_devices=num_devices, enable_asserts=enable_asserts)
    G = n_in // 8
    w2 = nc.dram_tensor("w2", [G, 8, E, JD], BF16, kind="ExternalInput").ap()
    xbd = nc.dram_tensor("xbd", [128, G, 64], BF16, kind="ExternalInput").ap()
    dsrep = nc.dram_tensor("dsrep", [128, 128], BF16, kind="ExternalInput").ap()
    d1rep = nc.dram_tensor("d1rep", [128, 128], BF16, kind="ExternalInput").ap()
    out = nc.dram_tensor("out", [b_loc, JD], F32, kind="ExternalOutput").ap()
    with tile.TileContext(nc) as tc:
        emit_capsule(tc, w2, xbd, dsrep, d1rep, out, n_in=n_in, b_loc=b_loc)
    nc.compile()
    return nc


def host_prep_w(weight, n_in):
    # weight [1, N, J, D, E] -> w2 [G, 8, E, J*D] with free layout (d, j)
    w2 = np.ascontiguousarray(weight[0].transpose(0, 3, 2, 1))  # [N, E, D, J]
    return w2.reshape(n_in // 8, 8, E, JD).astype(ml_dtypes.bfloat16)


def host_prep_xbd(xs, n_in):
    # xs [b_loc, N, E] -> xbd [128, G, 64] block-diagonal stationary, k-major
    G = n_in // 8
    t = xs.reshape(8, G, 8, E).transpose(1, 2, 3, 0)  # [G, di, e, b]
    xbd = np.zeros((G, 8, E, 8, 8), np.float32)       # [G, di, e, b, di']
    for di in range(8):
        xbd[:, di, :, :, di] = t[:, di]
    return np.ascontiguousarray(
        xbd.reshape(G, 128, 64).transpose(1, 0, 2)).astype(ml_dtypes.bfloat16)


def host_prep_deltas():
    p = np.arange(128)
    m = np.arange(128)
    mask = ((p[:, None] // 8) % 8) == ((m[None, :] // 8) % 8)
    dsrep = mask.astype(np.float32)
    d1rep = dsrep / 32.0
    return (dsrep.astype(ml_dtypes.bfloat16), d1rep.astype(ml_dtypes.bfloat16))


_CACHE = {}
LAST_EXEC_NS = None


def kernel(x, weight, trace=False):
    B, N_in = 64, 2048
    n_cores = 8
    b_loc = B // n_cores
    key = (N_in, b_loc, n_cores)
    if key not in _CACHE:
        _CACHE[key] = build_module(n_in=N_in, b_loc=b_loc, num_devices=n_cores)
    nc = _CACHE[key]

    x = np.asarray(x, dtype=np.float32)
    weight = np.asarray(weight, dtype=np.float32)
    w2 = host_prep_w(weight, N_in)
    dsrep, d1rep = host_prep_deltas()
    in_maps = []
    for c in range(n_cores):
        xs = np.ascontiguousarray(x[c * b_loc:(c + 1) * b_loc, :, 0, :])
        in_maps.append({
            "w2": w2,
            "xbd": host_prep_xbd(xs, N_in),
            "dsrep": dsrep,
            "d1rep": d1rep,
        })
    global LAST_EXEC_NS
    res = run_bass_kernel_spmd(nc, in_maps, core_ids=list(range(n_cores)),
                               trace=trace)
    LAST_EXEC_NS = res.exec_time_ns
    outs = [r["out"].reshape(b_loc, D, J).transpose(0, 2, 1) for r in res.results]
    return np.ascontiguousarray(np.concatenate(outs, axis=0))
